# revision 76
# baseline (speedup 1.0000x reference)
"""GQA attention block (RoPE + causal softmax + out-proj) on 8 TRN2 cores.

Sharding: 8 cores = 2 batches x 4 kv-pairs. Core c handles batch c//4 and
kv heads {2p, 2p+1} (p = c%4), i.e. q heads 6p..6p+5. Each core computes its
partial y^T = wo_slice^T @ attn_out^T; the host sums the 4 partials per batch
and transposes back.

Per-core layout: everything stays feature-major [d, s] so no on-device
transposes of large activations are needed:
  Q^T/K^T: [128d, s]   (projection emits them directly)
  scores come out transposed: [t, s] blocks from lhsT=K^T-slice, rhs=Q^T
  probs [t, s] feed AV directly with V in [t, dv] (via small PE transposes)
RoPE is applied in [d, s] form by permuting the head dim on the HOST to
[evens | odds]; the rotation becomes a partition-block swap (done with a PE
permutation matmul) plus elementwise mul/adds. The softmax scale is folded
into wq on the host. Softmax runs without max-subtraction (scores are O(10),
exp is safe in fp32).

Softmax denominators: probs tiles are accumulated on the vector engine into a
per-pair P_acc [128, 1024]; a single ones-matmul per pair half reduces it to
l, which is broadcast back with a tiny K=1 matmul and inverted on the DVE.
This removes the per-iteration row-sum matmuls from the PE.

The two q-head units of a pair share one [128, 1024] PSUM scores tile (two
banks) so a single ACT exp covers both. Diagonal blocks restrict the matmul /
exp column range to the causal suffix.

Emission is software-pipelined: the projection d-tile units of window j+1 and
the out-projection units of window j-1 are interleaved into window j's
attention loop as PE filler, so the PE never waits on the exp chain.
"""

import math
from collections import deque
from contextlib import ExitStack

import numpy as np
import ml_dtypes

import concourse.bass as bass
import concourse.mybir as mybir
import concourse.tile as tile
from concourse import bacc
from concourse.bass_utils import run_bass_kernel_spmd
from concourse.masks import make_identity

B, S, DIM = 2, 2048, 3072
NH, NKV, HD = 24, 8, 128
QT_PER_CORE = 6   # q head-tiles per core
KV_PER_CORE = 2   # kv heads per core
NDT = QT_PER_CORE + 2 * KV_PER_CORE  # 10 projection d-tiles
NKT = DIM // 128  # 24 contraction tiles
SW = 512          # s-window (matmul moving free dim)
NJ = S // SW      # 4 windows
NTT = S // 128    # 16 t-tiles
SCALE = 1.0 / math.sqrt(HD)

F32 = mybir.dt.float32
F32R = mybir.dt.float32r
BF16 = mybir.dt.bfloat16
BF = ml_dtypes.bfloat16

_PERM = np.concatenate([np.arange(0, HD, 2), np.arange(1, HD, 2)])

# projection d-tile order: k heads, v heads, then q tiles (so attention can
# start as early as possible in window 0)
DT_ORDER = [6, 7, 8, 9, 0, 1, 2, 3, 4, 5]


def _build_body(nc, tc, io, ctx):
    w10, wo4, yT = io["w10"], io["wo4"], io["yT"]
    x4 = io["x4"]
    ropeC, ropeS, masks, swp = io["ropeC"], io["ropeS"], io["masks"], io["swp"]

    singles = ctx.enter_context(tc.tile_pool(name="singles", bufs=1))
    ps = ctx.enter_context(tc.tile_pool(name="ps", bufs=1, space=bass.MemorySpace.PSUM))
    xt_pool = ctx.enter_context(tc.tile_pool(name="xtp", bufs=4))
    w_pool = ctx.enter_context(tc.tile_pool(name="wtp", bufs=4))
    wo_pool = ctx.enter_context(tc.tile_pool(name="wotp", bufs=4))
    raw_pool = ctx.enter_context(tc.tile_pool(name="rawp", bufs=3))
    qT_pool = ctx.enter_context(tc.tile_pool(name="qTp", bufs=12))
    pr_pool = ctx.enter_context(tc.tile_pool(name="prp", bufs=4))
    pacc_pool = ctx.enter_context(tc.tile_pool(name="paccp", bufs=2))
    small_pool = ctx.enter_context(tc.tile_pool(name="smp", bufs=2))
    out_pool = ctx.enter_context(tc.tile_pool(name="otp", bufs=18))
    y_pool = ctx.enter_context(tc.tile_pool(name="yp", bufs=3))

    # constants (const DMAs ride the gpsimd queue so they don't delay the
    # first x/weight loads)
    ropeC_sb = singles.tile([128, S], BF16, tag="ropeC", name="ropeC_sb")
    ropeS_sb = singles.tile([128, S], BF16, tag="ropeS", name="ropeS_sb")
    masks_sb = singles.tile([128, 4, SW], BF16, tag="masks", name="masks_sb")
    swp_sb = singles.tile([128, 128], BF16, tag="swp", name="swp_sb")
    ident = singles.tile([128, 128], F32, tag="ident", name="ident")
    ones32 = singles.tile([128, 1], F32R, tag="ones32", name="ones32")
    ones_mat = singles.tile([128, 128], F32R, tag="ones_mat", name="ones_mat")
    ones_mat0 = singles.tile([128, 128], F32, tag="ones_mat0", name="ones_mat0")
    # PE warm-up burst: keeps the HAM activity window busy from t~1us so the
    # clock gate is released (2.4 GHz) before the first real matmuls arrive,
    # and bridges the initial x/weight DMA ramp without going idle
    ww = singles.tile([128, 128], BF16, tag="ww", name="ww")
    nc.vector.memset(ww, 0.0)
    wps = ps.tile([128, 128], F32, tag="pp", bufs=2, name="wps")
    for _ in range(280):
        nc.tensor.matmul(wps, ww, ww, start=True, stop=True)

    nc.gpsimd.dma_start(out=ropeC_sb, in_=ropeC[:])
    nc.gpsimd.dma_start(out=ropeS_sb, in_=ropeS[:])
    nc.gpsimd.dma_start(out=masks_sb, in_=masks[:])
    nc.gpsimd.dma_start(out=swp_sb, in_=swp[:])
    make_identity(nc, ident)
    nc.vector.memset(ones_mat0, 1.0)
    nc.scalar.copy(out=ones_mat, in_=ones_mat0)
    nc.scalar.copy(out=ones32, in_=ones_mat0[:, 0:1])

    # per-window K^T / V tiles (written once by projections, read by attn)
    KT_w = [[singles.tile([128, SW], BF16, tag=f"KT{j}{g}", name=f"KT{j}{g}")
             for g in range(KV_PER_CORE)] for j in range(NJ)]
    V_w = [[singles.tile([128, 4, 128], BF16, tag=f"V{j}{g}", name=f"V{j}{g}")
            for g in range(KV_PER_CORE)] for j in range(NJ)]

    # window state
    xt_half = {}          # (j, h) -> tile [128, 12, SW]
    wt_tiles = {}         # (j, dt) -> weight tile [128, NKT, 128]
    wot_tiles = {}        # (j, d) -> wo tile [128, QT, 128]
    qT = [[None] * QT_PER_CORE for _ in range(NJ)]
    outT = [[None] * QT_PER_CORE for _ in range(NJ)]

    def dma_xt(j):
        a = xt_pool.tile([128, 12, SW], BF16, tag="xt", name="xt_a")
        b = xt_pool.tile([128, 12, SW], BF16, tag="xt", name="xt_b")
        nc.sync.dma_start(out=a, in_=x4[j, :, 0:12, :])
        nc.scalar.dma_start(out=b, in_=x4[j, :, 12:24, :])
        xt_half[(j, 0)], xt_half[(j, 1)] = a, b

    def dma_wt(j, n):
        # n-th projection weight tile (in DT_ORDER) for window j
        if n >= NDT:
            return
        dt = DT_ORDER[n]
        wt = w_pool.tile([128, NKT, 128], BF16, tag="wt", name="wt")
        if j == 0 and n >= 6:
            # window 0 is DMA-ramp-bound on the two HWDGE rings; route the
            # tail weight tiles through the otherwise-idle SWDGE path
            nc.gpsimd.dma_start(out=wt, in_=w10[dt])
        else:
            nc.sync.dma_start(out=wt[:, 0:12, :], in_=w10[dt, :, 0:12, :])
            nc.scalar.dma_start(out=wt[:, 12:24, :], in_=w10[dt, :, 12:24, :])
        wt_tiles[(j, dt)] = wt

    def dma_wot(j, d):
        if d >= NKT:
            return
        wot = wo_pool.tile([128, QT_PER_CORE, 128], BF16, tag="wot", name="wot")
        nc.sync.dma_start(out=wot, in_=wo4[d])
        wot_tiles[(j, d)] = wot

    # deferred post-processing (rope / V transpose) so the PE never waits on
    # the ACT evacuation of the projection PSUM
    post_q = deque()

    def emit_proj_unit(j, n):
        """Projection of d-tile DT_ORDER[n] for window j (24 PE matmuls)."""
        dt = DT_ORDER[n]
        dma_wt(j, n + 2)   # keep 2 tiles in flight
        jw = bass.ts(j, SW)
        wt = wt_tiles.pop((j, dt))
        xa, xb = xt_half[(j, 0)], xt_half[(j, 1)]
        pp = ps.tile([128, SW], F32, tag="pp", bufs=2, name="pp")
        for k in range(NKT):
            xs = xa[:, k, :] if k < 12 else xb[:, k - 12, :]
            nc.tensor.matmul(pp, wt[:, k, :], xs,
                             start=(k == 0), stop=(k == NKT - 1))
        if dt >= 8:
            g = dt - 8
            vraw = raw_pool.tile([128, SW], F32, tag="raw", name="vraw")
            nc.scalar.copy(out=vraw, in_=pp)

            def run_v(g=g, vraw=vraw, j=j):
                # deferred so the PE transposes never wait on the ACT evac
                tp = ps.tile([128, SW], F32, tag="pp", bufs=2, name="tp")
                for rr in range(4):
                    nc.tensor.transpose(tp[:, bass.ts(rr, 128)],
                                        vraw[:, bass.ts(rr, 128)], ident)
                nc.scalar.copy(out=V_w[j][g][:, 0:4, :],
                               in_=tp.rearrange("p (r t) -> p r t", r=4))
            post_q.append(run_v)
            if len(post_q) > 1:
                post_q.popleft()()
        else:
            raw = raw_pool.tile([128, SW], BF16, tag="raw", name="raw")
            nc.scalar.copy(out=raw, in_=pp)

            def run_qk(dt=dt, raw=raw, j=j, jw=jw):
                # deferred so the PE swap matmul never waits on the ACT evac
                sw_ps = ps.tile([128, SW], F32, tag="pp", bufs=2, name="sw_ps")
                nc.tensor.matmul(sw_ps, swp_sb, raw, start=True, stop=True)
                if dt < 6:
                    dest = qT_pool.tile([128, SW], BF16, tag="qt", name="qt")
                    qT[j][dt] = dest
                else:
                    dest = KT_w[j][dt - 6]
                nc.vector.tensor_mul(dest, raw, ropeC_sb[:, jw])
                t2 = raw_pool.tile([128, SW], BF16, tag="t2", name="t2")
                nc.vector.tensor_mul(t2, sw_ps, ropeS_sb[:, jw])
                nc.vector.tensor_add(dest, dest, t2)
            post_q.append(run_qk)
            if len(post_q) > 1:
                post_q.popleft()()

    def flush_post():
        while post_q:
            post_q.popleft()()

    oproj_state = {}

    def emit_oproj_half(j, d, half):
        """Half of out-projection d-tile d for window j (3 PE matmuls); the
        second half evacuates + stores. Split in two so the attention fill
        pacing gets finer granularity."""
        jw = bass.ts(j, SW)
        if half == 0:
            dma_wot(j, d + 3)
            wot = wot_tiles[(j, d)]
            yp = ps.tile([128, SW], F32, tag="pp", bufs=2, name="yp")
            oproj_state[(j, d)] = yp
            for u in range(3):
                nc.tensor.matmul(yp, wot[:, u, :], outT[j][u],
                                 start=(u == 0), stop=False,
                                 skip_group_check=True)
        else:
            wot = wot_tiles.pop((j, d))
            yp = oproj_state.pop((j, d))
            for u in range(3, QT_PER_CORE):
                nc.tensor.matmul(yp, wot[:, u, :], outT[j][u],
                                 start=False, stop=(u == QT_PER_CORE - 1),
                                 skip_group_check=True)
            ys = y_pool.tile([128, SW], BF16, tag="ys", name="ys")
            nc.vector.tensor_copy(out=ys, in_=yp)
            nc.scalar.dma_start(out=yT[bass.ts(d, 128), jw], in_=ys)

    # ---- fill-work queue (PE-dense units interleaved into attention) ----
    fills = deque()   # items: (kind, j, emit_thunk)

    def emit_attn_window(j):
        nlast = 4 * j + 3
        n_units = 3 * (4 * j + 4)
        qlen0 = len(fills)
        if j == NJ - 2:
            # leave some out-projection halves queued for the last window's
            # attention, which otherwise exhausts its PE filler near the end
            qlen0 = min(qlen0, 42)
        popped = 0
        unit_idx = 0
        for pair in range(QT_PER_CORE // 2):
            uA, uB = 2 * pair, 2 * pair + 1
            gA, gB = uA // 3, uB // 3
            qA, qB = qT[j][uA], qT[j][uB]
            av = ps.tile([128, 2 * SW], F32, tag="av", bufs=1, name="av")
            av_sb = small_pool.tile([128, 2 * SW], F32, tag="avsb", name="av_sb")
            pacc = pacc_pool.tile([128, 2 * SW], F32R, tag="pacc", name="pacc")
            for i in range(4 * j + 4):
                jj, r = divmod(i, 4)
                diag = (jj == j)
                c0 = 128 * (i - 4 * j) if diag else 0   # causal column cutoff
                sc = ps.tile([128, 2 * SW], F32, tag="sc", bufs=2, name="sc")
                nc.tensor.matmul(sc[:, c0:SW], KT_w[jj][gA][:, bass.ts(r, 128)],
                                 qA[:, c0:SW], start=True, stop=True)
                nc.tensor.matmul(sc[:, SW + c0:2 * SW],
                                 KT_w[jj][gB][:, bass.ts(r, 128)],
                                 qB[:, c0:SW], start=True, stop=True)
                pr = pr_pool.tile([128, 2 * SW], BF16, tag="pr", name="pr")
                if c0 == 0:
                    nc.scalar.activation(out=pr, in_=sc,
                                         func=mybir.ActivationFunctionType.Exp)
                else:
                    nc.scalar.activation(out=pr[:, c0:SW], in_=sc[:, c0:SW],
                                         func=mybir.ActivationFunctionType.Exp)
                    nc.scalar.activation(out=pr[:, SW + c0:2 * SW],
                                         in_=sc[:, SW + c0:2 * SW],
                                         func=mybir.ActivationFunctionType.Exp)
                if diag:
                    # mask zeroes the strict upper triangle of the diagonal
                    # block; ops stay within the written column suffix so no
                    # stale SBUF is ever read
                    nc.vector.tensor_mul(pr[:, c0:SW], pr[:, c0:SW],
                                         masks_sb[:, r, c0:SW])
                    nc.vector.tensor_mul(pr[:, SW + c0:2 * SW],
                                         pr[:, SW + c0:2 * SW],
                                         masks_sb[:, r, c0:SW])
                if i == 0:
                    nc.vector.tensor_copy(out=pacc, in_=pr)
                elif c0 == 0:
                    nc.vector.tensor_add(pacc, pacc, pr)
                else:
                    nc.vector.tensor_add(pacc[:, c0:SW], pacc[:, c0:SW],
                                         pr[:, c0:SW])
                    nc.vector.tensor_add(pacc[:, SW + c0:2 * SW],
                                         pacc[:, SW + c0:2 * SW],
                                         pr[:, SW + c0:2 * SW])
                nc.tensor.matmul(av[:, c0:SW], V_w[jj][gA][:, r, :],
                                 pr[:, c0:SW], start=(i == 0), stop=(i == nlast),
                                 skip_group_check=True)
                if i == nlast:
                    # evacuate the A half while the B half's matmul still runs
                    nc.vector.tensor_copy(out=av_sb[:, 0:SW], in_=av[:, 0:SW])
                nc.tensor.matmul(av[:, SW + c0:2 * SW], V_w[jj][gB][:, r, :],
                                 pr[:, SW + c0:2 * SW],
                                 start=(i == 0), stop=(i == nlast),
                                 skip_group_check=True)
                if i == nlast:
                    nc.vector.tensor_copy(out=av_sb[:, SW:2 * SW],
                                          in_=av[:, SW:2 * SW])
                # interleave queued PE-dense fill work, evenly over the window
                unit_idx += 1
                target = min(qlen0, (qlen0 * unit_idx + n_units - 1) // n_units)
                while popped < target:
                    fills.popleft()[2]()
                    popped += 1
            # ---- pair finalize: l = rowsum(P_acc), outT = av / l ----
            # evacuate av to SBUF right away so its PSUM banks free for the
            # next pair without waiting on the whole normalization chain
            # the finalize matmuls wait on the DVE reduction chain; fills are
            # safe to place here now that lp/rb use the sc tag (no pp-slot
            # coupling to fill units)
            extra = 0
            while fills and extra < 2:
                fills.popleft()[2]()
                popped += 1
                extra += 1
            for a, u in ((0, uA), (1, uB)):
                # lp/rb live in the sc tag: its slots free right after exp, so
                # the finalize never waits behind interleaved fill work (the
                # pp tag couples to projection/out-projection units)
                lp = ps.tile([128, SW], F32, tag="sc", bufs=2, name="lp")
                nc.tensor.matmul(lp[0:1, :], ones32, pacc[:, bass.ts(a, SW)],
                                 start=True, stop=True)
                l_sbp = small_pool.tile([1, SW], F32R, tag="l_sbp", name="l_sbp")
                nc.scalar.copy(out=l_sbp[0:1, :], in_=lp[0:1, :])
                rb = ps.tile([128, SW], F32, tag="sc", bufs=2, name="rb")
                nc.tensor.matmul(rb, ones_mat[0:1, :], l_sbp[0:1, :],
                                 start=True, stop=True)
                rbs = small_pool.tile([128, SW], F32, tag="rbs", name="rbs")
                nc.vector.reciprocal_approx_fast(out=rbs, in_=rb)
                ot = out_pool.tile([128, SW], BF16, tag="ot", name="ot")
                nc.vector.tensor_mul(ot, av_sb[:, bass.ts(a, SW)], rbs)
                outT[j][u] = ot

    # =================== emission =====================
    # startup: first x window (fine slices, both queues) + first weights
    dma_wt(0, 0)
    xa0 = xt_pool.tile([128, 12, SW], BF16, tag="xt", name="xt_a")
    xb0 = xt_pool.tile([128, 12, SW], BF16, tag="xt", name="xt_b")
    xsl = [(0, 1), (1, 2), (2, 4), (4, 6), (6, 9), (9, 12)]
    for n, (k0, k1) in enumerate(xsl):
        eng = nc.sync if n % 2 == 0 else nc.scalar
        eng.dma_start(out=xa0[:, k0:k1, :], in_=x4[0, :, k0:k1, :])
    for n, (k0, k1) in enumerate(xsl):
        eng = nc.scalar if n % 2 == 0 else nc.sync
        eng.dma_start(out=xb0[:, k0:k1, :], in_=x4[0, :, (12 + k0):(12 + k1), :])
    xt_half[(0, 0)], xt_half[(0, 1)] = xa0, xb0
    dma_wt(0, 1)

    # window 0 projections run solid (nothing to interleave yet)
    for n in range(NDT):
        emit_proj_unit(0, n)
    flush_post()

    for j in range(NJ):
        # drain any leftover projection fills for THIS window (attention
        # depends on them); oproj fills of earlier windows stay queued
        if any(f[0] == "proj" and f[1] == j for f in fills):
            rest = deque()
            while fills:
                item = fills.popleft()
                if item[0] == "proj" and item[1] == j:
                    item[2]()
                else:
                    rest.append(item)
            fills.extend(rest)
        flush_post()
        if j < NJ - 1:
            # stage next window's inputs + queue its projection units
            dma_xt(j + 1)
            dma_wt(j + 1, 0)
            dma_wt(j + 1, 1)
            for n in range(NDT):
                fills.append(("proj", j + 1,
                              (lambda jj, nn: lambda: emit_proj_unit(jj, nn))(j + 1, n)))
        emit_attn_window(j)
        flush_post()
        # queue this window's out-projection as fill work for the next
        # window's attention
        dma_wot(j, 0)
        dma_wot(j, 1)
        dma_wot(j, 2)
        for d in range(NKT):
            for h in range(2):
                fills.append(("oproj", j,
                              (lambda jj, dd, hh: lambda: emit_oproj_half(jj, dd, hh))(j, d, h)))
    while fills:
        fills.popleft()[2]()
    flush_post()


def build_nc():
    nc = bacc.Bacc("TRN2", target_bir_lowering=False, debug=False, num_devices=8)
    io = {
        "x4": nc.dram_tensor("x4", [NJ, 128, NKT, SW], BF16, kind="ExternalInput"),
        "w10": nc.dram_tensor("w10", [NDT, 128, NKT, 128], BF16, kind="ExternalInput"),
        "wo4": nc.dram_tensor("wo4", [NKT, 128, QT_PER_CORE, 128], BF16,
                              kind="ExternalInput"),
        "ropeC": nc.dram_tensor("ropeC", [HD, S], BF16, kind="ExternalInput"),
        "ropeS": nc.dram_tensor("ropeS", [HD, S], BF16, kind="ExternalInput"),
        "masks": nc.dram_tensor("masks", [128, 4, SW], BF16, kind="ExternalInput"),
        "swp": nc.dram_tensor("swp", [128, 128], BF16, kind="ExternalInput"),
        "yT": nc.dram_tensor("yT", [DIM, S], BF16, kind="ExternalOutput"),
    }
    with tile.TileContext(nc) as tc:
        with ExitStack() as ctx:
            _build_body(nc, tc, io, ctx)
    nc.compile()
    return nc


_NC = None


def _get_nc():
    global _NC
    if _NC is None:
        _NC = build_nc()
    return _NC


def make_in_maps(x, wq, wk, wv, wo, freqs_cos, freqs_sin):
    x = np.asarray(x, np.float32)
    wq = np.asarray(wq, np.float32)
    wk = np.asarray(wk, np.float32)
    wv = np.asarray(wv, np.float32)
    wo = np.asarray(wo, np.float32)
    cos = np.asarray(freqs_cos, np.float32)
    sin = np.asarray(freqs_sin, np.float32)

    wq_p = (wq.reshape(DIM, NH, HD)[:, :, _PERM] * SCALE).astype(BF)
    wk_p = wk.reshape(DIM, NKV, HD)[:, :, _PERM].astype(BF)
    wv_r = wv.reshape(DIM, NKV, HD).astype(BF)
    wo_r = wo.reshape(NH, HD, DIM)

    ropeC = np.ascontiguousarray(np.concatenate([cos.T, cos.T], 0)).astype(BF)
    ropeS = np.ascontiguousarray(np.concatenate([-sin.T, sin.T], 0)).astype(BF)

    tt = np.arange(128)[:, None]
    ss = np.arange(SW)[None, :]
    # [128, 4, SW] with masks[:, r, :] the r-th diagonal-block pattern
    masks = np.stack([(128 * r + tt <= ss) for r in range(4)], axis=1).astype(BF)

    swp = np.zeros((128, 128), BF)
    swp[np.arange(128), (np.arange(128) + 64) % 128] = 1.0

    in_maps = []
    for c in range(8):
        b, p = divmod(c, 4)
        # per-core weight slices in on-chip tile layout
        wq_c = wq_p[:, 6 * p:6 * p + 6, :]          # [DIM, 6, 128]
        wk_c = wk_p[:, 2 * p:2 * p + 2, :]          # [DIM, 2, 128]
        wv_c = wv_r[:, 2 * p:2 * p + 2, :]          # [DIM, 2, 128]
        # w10[dt] = [128p, 24k, 128d] with DIM rows split as (k, p)
        wcat = np.concatenate([wq_c, wk_c, wv_c], axis=1)   # [DIM, 10, 128]
        w10 = np.ascontiguousarray(
            wcat.reshape(NKT, 128, NDT, HD).transpose(2, 1, 0, 3))
        # wo4[d] = [128p(dv), 6u, 128dd]; wo rows are (u, p)
        wo_c = wo_r[6 * p:6 * p + 6]                 # [6, 128, DIM]
        wo4 = np.ascontiguousarray(
            wo_c.reshape(QT_PER_CORE, HD, NKT, 128).transpose(2, 1, 0, 3)).astype(BF)
        # x4[j] = [128p, 24k, 512s]
        xT_b = x[b].T                                 # [DIM, S]
        x4 = np.ascontiguousarray(
            xT_b.reshape(NKT, 128, NJ, SW).transpose(2, 1, 0, 3)).astype(BF)
        in_maps.append({
            "x4": x4,
            "w10": w10,
            "wo4": wo4,
            "ropeC": ropeC,
            "ropeS": ropeS,
            "masks": masks,
            "swp": swp,
        })
    return in_maps


def gather(results):
    y = np.empty((B, S, DIM), np.float32)
    for b in range(B):
        acc = results[4 * b]["yT"].astype(np.float32)
        for p in range(1, 4):
            acc = acc + results[4 * b + p]["yT"].astype(np.float32)
        y[b] = acc.T
    return y


def kernel(x, wq, wk, wv, wo, freqs_cos, freqs_sin, **run_kwargs):
    nc = _get_nc()
    in_maps = make_in_maps(x, wq, wk, wv, wo, freqs_cos, freqs_sin)
    res = run_bass_kernel_spmd(nc, in_maps, core_ids=list(range(8)), **run_kwargs)
    out = gather(res.results)
    if run_kwargs:
        return out, res
    return out


# revision 77
# speedup vs baseline: 1.0227x; 1.0227x over previous
"""GQA attention block (RoPE + causal softmax + out-proj) on 8 TRN2 cores.

Sharding: 8 cores = 2 batches x 4 kv-pairs. Core c handles batch c//4 and
kv heads {2p, 2p+1} (p = c%4), i.e. q heads 6p..6p+5. Each core computes its
partial y^T = wo_slice^T @ attn_out^T; the host sums the 4 partials per batch
and transposes back.

Per-core layout: everything stays feature-major [d, s] so no on-device
transposes of large activations are needed:
  Q^T/K^T: [128d, s]   (projection emits them directly)
  scores come out transposed: [t, s] blocks from lhsT=K^T-slice, rhs=Q^T
  probs [t, s] feed AV directly with V in [t, dv] (via small PE transposes)
RoPE is applied in [d, s] form by permuting the head dim on the HOST to
[evens | odds]; the rotation becomes a partition-block swap (done with a PE
permutation matmul) plus elementwise mul/adds. The softmax scale is folded
into wq on the host. Softmax runs without max-subtraction (scores are O(10),
exp is safe in fp32).

Softmax denominators: probs tiles are accumulated on the vector engine into a
per-pair P_acc [128, 1024]; a single ones-matmul per pair half reduces it to
l, which is broadcast back with a tiny K=1 matmul and inverted on the DVE.
This removes the per-iteration row-sum matmuls from the PE.

The two q-head units of a pair share one [128, 1024] PSUM scores tile (two
banks) so a single ACT exp covers both. Diagonal blocks restrict the matmul /
exp column range to the causal suffix.

Emission is software-pipelined: the projection d-tile units of window j+1 and
the out-projection units of window j-1 are interleaved into window j's
attention loop as PE filler, so the PE never waits on the exp chain.
"""

import math
from collections import deque
from contextlib import ExitStack

import numpy as np
import ml_dtypes

import concourse.bass as bass
import concourse.mybir as mybir
import concourse.tile as tile
from concourse import bacc
from concourse.bass_utils import run_bass_kernel_spmd
from concourse.masks import make_identity

B, S, DIM = 2, 2048, 3072
NH, NKV, HD = 24, 8, 128
QT_PER_CORE = 6   # q head-tiles per core
KV_PER_CORE = 2   # kv heads per core
NDT = QT_PER_CORE + 2 * KV_PER_CORE  # 10 projection d-tiles
NKT = DIM // 128  # 24 contraction tiles
SW = 512          # s-window (matmul moving free dim)
NJ = S // SW      # 4 windows
NTT = S // 128    # 16 t-tiles
SCALE = 1.0 / math.sqrt(HD)

F32 = mybir.dt.float32
F32R = mybir.dt.float32r
BF16 = mybir.dt.bfloat16
BF = ml_dtypes.bfloat16

_PERM = np.concatenate([np.arange(0, HD, 2), np.arange(1, HD, 2)])

# projection d-tile order: k heads, v heads, then q tiles (so attention can
# start as early as possible in window 0)
DT_ORDER = [6, 7, 8, 9, 0, 1, 2, 3, 4, 5]


def _build_body(nc, tc, io, ctx):
    w10, wo4, yT = io["w10"], io["wo4"], io["yT"]
    x4 = io["x4"]
    ropeC, ropeS, masks, swp = io["ropeC"], io["ropeS"], io["masks"], io["swp"]

    singles = ctx.enter_context(tc.tile_pool(name="singles", bufs=1))
    ps = ctx.enter_context(tc.tile_pool(name="ps", bufs=1, space=bass.MemorySpace.PSUM))
    xt_pool = ctx.enter_context(tc.tile_pool(name="xtp", bufs=4))
    w_pool = ctx.enter_context(tc.tile_pool(name="wtp", bufs=4))
    wo_pool = ctx.enter_context(tc.tile_pool(name="wotp", bufs=4))
    raw_pool = ctx.enter_context(tc.tile_pool(name="rawp", bufs=3))
    qT_pool = ctx.enter_context(tc.tile_pool(name="qTp", bufs=12))
    pr_pool = ctx.enter_context(tc.tile_pool(name="prp", bufs=4))
    pacc_pool = ctx.enter_context(tc.tile_pool(name="paccp", bufs=2))
    small_pool = ctx.enter_context(tc.tile_pool(name="smp", bufs=2))
    out_pool = ctx.enter_context(tc.tile_pool(name="otp", bufs=18))
    y_pool = ctx.enter_context(tc.tile_pool(name="yp", bufs=3))

    # constants (const DMAs ride the gpsimd queue so they don't delay the
    # first x/weight loads)
    ropeC_sb = singles.tile([128, S], BF16, tag="ropeC", name="ropeC_sb")
    ropeS_sb = singles.tile([128, S], BF16, tag="ropeS", name="ropeS_sb")
    masks_sb = singles.tile([128, 4, SW], BF16, tag="masks", name="masks_sb")
    swp_sb = singles.tile([128, 128], BF16, tag="swp", name="swp_sb")
    ident = singles.tile([128, 128], F32, tag="ident", name="ident")
    ones32 = singles.tile([128, 1], F32R, tag="ones32", name="ones32")
    ones_mat = singles.tile([128, 128], F32R, tag="ones_mat", name="ones_mat")
    ones_mat0 = singles.tile([128, 128], F32, tag="ones_mat0", name="ones_mat0")
    # PE warm-up burst: keeps the HAM activity window busy from t~1us so the
    # clock gate is released (2.4 GHz) before the first real matmuls arrive,
    # and bridges the initial x/weight DMA ramp without going idle
    ww = singles.tile([128, 128], BF16, tag="ww", name="ww")
    nc.vector.memset(ww, 0.0)
    wps = ps.tile([128, 128], F32, tag="pp", bufs=2, name="wps")
    for _ in range(280):
        nc.tensor.matmul(wps, ww, ww, start=True, stop=True)

    nc.gpsimd.dma_start(out=ropeC_sb, in_=ropeC[:])
    nc.gpsimd.dma_start(out=ropeS_sb, in_=ropeS[:])
    nc.gpsimd.dma_start(out=masks_sb, in_=masks[:])
    nc.gpsimd.dma_start(out=swp_sb, in_=swp[:])
    make_identity(nc, ident)
    nc.vector.memset(ones_mat0, 1.0)
    nc.scalar.copy(out=ones_mat, in_=ones_mat0)
    nc.scalar.copy(out=ones32, in_=ones_mat0[:, 0:1])

    # per-window K^T / V tiles (written once by projections, read by attn)
    KT_w = [[singles.tile([128, SW], BF16, tag=f"KT{j}{g}", name=f"KT{j}{g}")
             for g in range(KV_PER_CORE)] for j in range(NJ)]
    V_w = [[singles.tile([128, 4, 128], BF16, tag=f"V{j}{g}", name=f"V{j}{g}")
            for g in range(KV_PER_CORE)] for j in range(NJ)]

    # window state
    xt_half = {}          # (j, h) -> tile [128, 12, SW]
    wt_tiles = {}         # (j, dt) -> weight tile [128, NKT, 128]
    wot_tiles = {}        # (j, d) -> wo tile [128, QT, 128]
    qT = [[None] * QT_PER_CORE for _ in range(NJ)]
    outT = [[None] * QT_PER_CORE for _ in range(NJ)]

    def dma_xt(j):
        a = xt_pool.tile([128, 12, SW], BF16, tag="xt", name="xt_a")
        b = xt_pool.tile([128, 12, SW], BF16, tag="xt", name="xt_b")
        nc.sync.dma_start(out=a, in_=x4[j, :, 0:12, :])
        nc.scalar.dma_start(out=b, in_=x4[j, :, 12:24, :])
        xt_half[(j, 0)], xt_half[(j, 1)] = a, b

    def dma_wt(j, n):
        # n-th projection weight tile (in DT_ORDER) for window j
        if n >= NDT:
            return
        dt = DT_ORDER[n]
        wt = w_pool.tile([128, NKT, 128], BF16, tag="wt", name="wt")
        if j == 0 and n >= 6:
            # window 0 is DMA-ramp-bound on the two HWDGE rings; route the
            # tail weight tiles through the otherwise-idle SWDGE path
            nc.gpsimd.dma_start(out=wt, in_=w10[dt])
        else:
            nc.sync.dma_start(out=wt[:, 0:12, :], in_=w10[dt, :, 0:12, :])
            nc.scalar.dma_start(out=wt[:, 12:24, :], in_=w10[dt, :, 12:24, :])
        wt_tiles[(j, dt)] = wt

    def dma_wot(j, d):
        if d >= NKT:
            return
        wot = wo_pool.tile([128, QT_PER_CORE, 128], BF16, tag="wot", name="wot")
        nc.sync.dma_start(out=wot, in_=wo4[d])
        wot_tiles[(j, d)] = wot

    # deferred post-processing (rope / V transpose) so the PE never waits on
    # the ACT evacuation of the projection PSUM
    post_q = deque()

    def emit_proj_unit(j, n):
        """Projection of d-tile DT_ORDER[n] for window j (24 PE matmuls)."""
        dt = DT_ORDER[n]
        dma_wt(j, n + 2)   # keep 2 tiles in flight
        jw = bass.ts(j, SW)
        wt = wt_tiles.pop((j, dt))
        xa, xb = xt_half[(j, 0)], xt_half[(j, 1)]
        pp = ps.tile([128, SW], F32, tag="pp", bufs=2, name="pp")
        for k in range(NKT):
            xs = xa[:, k, :] if k < 12 else xb[:, k - 12, :]
            nc.tensor.matmul(pp, wt[:, k, :], xs,
                             start=(k == 0), stop=(k == NKT - 1))
        if dt >= 8:
            g = dt - 8
            vraw = raw_pool.tile([128, SW], F32, tag="raw", name="vraw")
            nc.scalar.copy(out=vraw, in_=pp)

            def run_v(g=g, vraw=vraw, j=j):
                # deferred so the PE transposes never wait on the ACT evac
                tp = ps.tile([128, SW], F32, tag="pp", bufs=2, name="tp")
                for rr in range(4):
                    nc.tensor.transpose(tp[:, bass.ts(rr, 128)],
                                        vraw[:, bass.ts(rr, 128)], ident)
                nc.scalar.copy(out=V_w[j][g][:, 0:4, :],
                               in_=tp.rearrange("p (r t) -> p r t", r=4))
            post_q.append(run_v)
            if len(post_q) > 1:
                post_q.popleft()()
        else:
            raw = raw_pool.tile([128, SW], BF16, tag="raw", name="raw")
            nc.scalar.copy(out=raw, in_=pp)

            def run_qk(dt=dt, raw=raw, j=j, jw=jw):
                # deferred so the PE swap matmul never waits on the ACT evac
                sw_ps = ps.tile([128, SW], F32, tag="pp", bufs=2, name="sw_ps")
                nc.tensor.matmul(sw_ps, swp_sb, raw, start=True, stop=True)
                if dt < 6:
                    dest = qT_pool.tile([128, SW], BF16, tag="qt", name="qt")
                    qT[j][dt] = dest
                else:
                    dest = KT_w[j][dt - 6]
                nc.vector.tensor_mul(dest, raw, ropeC_sb[:, jw])
                t2 = raw_pool.tile([128, SW], BF16, tag="t2", name="t2")
                nc.vector.tensor_mul(t2, sw_ps, ropeS_sb[:, jw])
                nc.vector.tensor_add(dest, dest, t2)
            post_q.append(run_qk)
            if len(post_q) > 1:
                post_q.popleft()()

    def flush_post():
        while post_q:
            post_q.popleft()()

    oproj_state = {}

    def emit_oproj_half(j, d, half):
        """Half of out-projection d-tile d for window j (3 PE matmuls); the
        second half evacuates + stores. Split in two so the attention fill
        pacing gets finer granularity."""
        jw = bass.ts(j, SW)
        if half == 0:
            dma_wot(j, d + 3)
            wot = wot_tiles[(j, d)]
            yp = ps.tile([128, SW], F32, tag="pp", bufs=2, name="yp")
            oproj_state[(j, d)] = yp
            for u in range(3):
                nc.tensor.matmul(yp, wot[:, u, :], outT[j][u],
                                 start=(u == 0), stop=False,
                                 skip_group_check=True)
        else:
            wot = wot_tiles.pop((j, d))
            yp = oproj_state.pop((j, d))
            for u in range(3, QT_PER_CORE):
                nc.tensor.matmul(yp, wot[:, u, :], outT[j][u],
                                 start=False, stop=(u == QT_PER_CORE - 1),
                                 skip_group_check=True)
            ys = y_pool.tile([128, SW], BF16, tag="ys", name="ys")
            nc.vector.tensor_copy(out=ys, in_=yp)
            nc.scalar.dma_start(out=yT[bass.ts(d, 128), jw], in_=ys)

    # ---- fill-work queue (PE-dense units interleaved into attention) ----
    fills = deque()   # items: (kind, j, emit_thunk)

    def emit_attn_window(j):
        nlast = 4 * j + 3
        n_units = 3 * (4 * j + 4)
        qlen0 = len(fills)
        popped = 0
        unit_idx = 0
        for pair in range(QT_PER_CORE // 2):
            uA, uB = 2 * pair, 2 * pair + 1
            gA, gB = uA // 3, uB // 3
            qA, qB = qT[j][uA], qT[j][uB]
            av = ps.tile([128, 2 * SW], F32, tag="av", bufs=1, name="av")
            av_sb = small_pool.tile([128, 2 * SW], F32, tag="avsb", name="av_sb")
            pacc = pacc_pool.tile([128, 2 * SW], F32R, tag="pacc", name="pacc")
            for i in range(4 * j + 4):
                jj, r = divmod(i, 4)
                diag = (jj == j)
                c0 = 128 * (i - 4 * j) if diag else 0   # causal column cutoff
                sc = ps.tile([128, 2 * SW], F32, tag="sc", bufs=2, name="sc")
                nc.tensor.matmul(sc[:, c0:SW], KT_w[jj][gA][:, bass.ts(r, 128)],
                                 qA[:, c0:SW], start=True, stop=True)
                nc.tensor.matmul(sc[:, SW + c0:2 * SW],
                                 KT_w[jj][gB][:, bass.ts(r, 128)],
                                 qB[:, c0:SW], start=True, stop=True)
                pr = pr_pool.tile([128, 2 * SW], BF16, tag="pr", name="pr")
                if c0 == 0:
                    nc.scalar.activation(out=pr, in_=sc,
                                         func=mybir.ActivationFunctionType.Exp)
                else:
                    nc.scalar.activation(out=pr[:, c0:SW], in_=sc[:, c0:SW],
                                         func=mybir.ActivationFunctionType.Exp)
                    nc.scalar.activation(out=pr[:, SW + c0:2 * SW],
                                         in_=sc[:, SW + c0:2 * SW],
                                         func=mybir.ActivationFunctionType.Exp)
                if diag:
                    # mask zeroes the strict upper triangle of the diagonal
                    # block; ops stay within the written column suffix so no
                    # stale SBUF is ever read
                    nc.vector.tensor_mul(pr[:, c0:SW], pr[:, c0:SW],
                                         masks_sb[:, r, c0:SW])
                    nc.vector.tensor_mul(pr[:, SW + c0:2 * SW],
                                         pr[:, SW + c0:2 * SW],
                                         masks_sb[:, r, c0:SW])
                if i == 0:
                    nc.vector.tensor_copy(out=pacc, in_=pr)
                elif c0 == 0:
                    nc.vector.tensor_add(pacc, pacc, pr)
                else:
                    nc.vector.tensor_add(pacc[:, c0:SW], pacc[:, c0:SW],
                                         pr[:, c0:SW])
                    nc.vector.tensor_add(pacc[:, SW + c0:2 * SW],
                                         pacc[:, SW + c0:2 * SW],
                                         pr[:, SW + c0:2 * SW])
                nc.tensor.matmul(av[:, c0:SW], V_w[jj][gA][:, r, :],
                                 pr[:, c0:SW], start=(i == 0), stop=(i == nlast),
                                 skip_group_check=True)
                if i == nlast:
                    # evacuate the A half while the B half's matmul still runs
                    nc.vector.tensor_copy(out=av_sb[:, 0:SW], in_=av[:, 0:SW])
                nc.tensor.matmul(av[:, SW + c0:2 * SW], V_w[jj][gB][:, r, :],
                                 pr[:, SW + c0:2 * SW],
                                 start=(i == 0), stop=(i == nlast),
                                 skip_group_check=True)
                if i == nlast:
                    nc.vector.tensor_copy(out=av_sb[:, SW:2 * SW],
                                          in_=av[:, SW:2 * SW])
                # interleave queued PE-dense fill work, evenly over the window
                unit_idx += 1
                target = min(qlen0, (qlen0 * unit_idx + n_units - 1) // n_units)
                while popped < target:
                    fills.popleft()[2]()
                    popped += 1
            # ---- pair finalize: l = rowsum(P_acc), outT = av / l ----
            # evacuate av to SBUF right away so its PSUM banks free for the
            # next pair without waiting on the whole normalization chain
            # the finalize matmuls wait on the DVE reduction chain; fills are
            # safe to place here now that lp/rb use the sc tag (no pp-slot
            # coupling to fill units)
            extra = 0
            while fills and extra < 2:
                fills.popleft()[2]()
                popped += 1
                extra += 1
            for a, u in ((0, uA), (1, uB)):
                # lp/rb live in the sc tag: its slots free right after exp, so
                # the finalize never waits behind interleaved fill work (the
                # pp tag couples to projection/out-projection units)
                lp = ps.tile([128, SW], F32, tag="sc", bufs=2, name="lp")
                nc.tensor.matmul(lp[0:1, :], ones32, pacc[:, bass.ts(a, SW)],
                                 start=True, stop=True)
                l_sbp = small_pool.tile([1, SW], F32R, tag="l_sbp", name="l_sbp")
                nc.scalar.copy(out=l_sbp[0:1, :], in_=lp[0:1, :])
                rb = ps.tile([128, SW], F32, tag="sc", bufs=2, name="rb")
                nc.tensor.matmul(rb, ones_mat[0:1, :], l_sbp[0:1, :],
                                 start=True, stop=True)
                rbs = small_pool.tile([128, SW], F32, tag="rbs", name="rbs")
                nc.vector.reciprocal_approx_fast(out=rbs, in_=rb)
                ot = out_pool.tile([128, SW], BF16, tag="ot", name="ot")
                nc.vector.tensor_mul(ot, av_sb[:, bass.ts(a, SW)], rbs)
                outT[j][u] = ot

    # =================== emission =====================
    # startup: first x window (fine slices, both queues) + first weights
    dma_wt(0, 0)
    xa0 = xt_pool.tile([128, 12, SW], BF16, tag="xt", name="xt_a")
    xb0 = xt_pool.tile([128, 12, SW], BF16, tag="xt", name="xt_b")
    xsl = [(0, 1), (1, 2), (2, 4), (4, 6), (6, 9), (9, 12)]
    for n, (k0, k1) in enumerate(xsl):
        eng = nc.sync if n % 2 == 0 else nc.scalar
        eng.dma_start(out=xa0[:, k0:k1, :], in_=x4[0, :, k0:k1, :])
    for n, (k0, k1) in enumerate(xsl):
        eng = nc.scalar if n % 2 == 0 else nc.sync
        eng.dma_start(out=xb0[:, k0:k1, :], in_=x4[0, :, (12 + k0):(12 + k1), :])
    xt_half[(0, 0)], xt_half[(0, 1)] = xa0, xb0
    dma_wt(0, 1)

    # window 0 projections run solid (nothing to interleave yet)
    for n in range(NDT):
        emit_proj_unit(0, n)
    flush_post()

    for j in range(NJ):
        # drain any leftover projection fills for THIS window (attention
        # depends on them); oproj fills of earlier windows stay queued
        if any(f[0] == "proj" and f[1] == j for f in fills):
            rest = deque()
            while fills:
                item = fills.popleft()
                if item[0] == "proj" and item[1] == j:
                    item[2]()
                else:
                    rest.append(item)
            fills.extend(rest)
        flush_post()
        if j < NJ - 1:
            # stage next window's inputs + queue its projection units
            dma_xt(j + 1)
            dma_wt(j + 1, 0)
            dma_wt(j + 1, 1)
            for n in range(NDT):
                fills.append(("proj", j + 1,
                              (lambda jj, nn: lambda: emit_proj_unit(jj, nn))(j + 1, n)))
        emit_attn_window(j)
        flush_post()
        # queue this window's out-projection as fill work for the next
        # window's attention
        dma_wot(j, 0)
        dma_wot(j, 1)
        dma_wot(j, 2)
        for d in range(NKT):
            for h in range(2):
                fills.append(("oproj", j,
                              (lambda jj, dd, hh: lambda: emit_oproj_half(jj, dd, hh))(j, d, h)))
    while fills:
        fills.popleft()[2]()
    flush_post()


def build_nc():
    nc = bacc.Bacc("TRN2", target_bir_lowering=False, debug=False, num_devices=8)
    io = {
        "x4": nc.dram_tensor("x4", [NJ, 128, NKT, SW], BF16, kind="ExternalInput"),
        "w10": nc.dram_tensor("w10", [NDT, 128, NKT, 128], BF16, kind="ExternalInput"),
        "wo4": nc.dram_tensor("wo4", [NKT, 128, QT_PER_CORE, 128], BF16,
                              kind="ExternalInput"),
        "ropeC": nc.dram_tensor("ropeC", [HD, S], BF16, kind="ExternalInput"),
        "ropeS": nc.dram_tensor("ropeS", [HD, S], BF16, kind="ExternalInput"),
        "masks": nc.dram_tensor("masks", [128, 4, SW], BF16, kind="ExternalInput"),
        "swp": nc.dram_tensor("swp", [128, 128], BF16, kind="ExternalInput"),
        "yT": nc.dram_tensor("yT", [DIM, S], BF16, kind="ExternalOutput"),
    }
    with tile.TileContext(nc) as tc:
        with ExitStack() as ctx:
            _build_body(nc, tc, io, ctx)
    nc.compile()
    return nc


_NC = None


def _get_nc():
    global _NC
    if _NC is None:
        _NC = build_nc()
    return _NC


def make_in_maps(x, wq, wk, wv, wo, freqs_cos, freqs_sin):
    x = np.asarray(x, np.float32)
    wq = np.asarray(wq, np.float32)
    wk = np.asarray(wk, np.float32)
    wv = np.asarray(wv, np.float32)
    wo = np.asarray(wo, np.float32)
    cos = np.asarray(freqs_cos, np.float32)
    sin = np.asarray(freqs_sin, np.float32)

    wq_p = (wq.reshape(DIM, NH, HD)[:, :, _PERM] * SCALE).astype(BF)
    wk_p = wk.reshape(DIM, NKV, HD)[:, :, _PERM].astype(BF)
    wv_r = wv.reshape(DIM, NKV, HD).astype(BF)
    wo_r = wo.reshape(NH, HD, DIM)

    ropeC = np.ascontiguousarray(np.concatenate([cos.T, cos.T], 0)).astype(BF)
    ropeS = np.ascontiguousarray(np.concatenate([-sin.T, sin.T], 0)).astype(BF)

    tt = np.arange(128)[:, None]
    ss = np.arange(SW)[None, :]
    # [128, 4, SW] with masks[:, r, :] the r-th diagonal-block pattern
    masks = np.stack([(128 * r + tt <= ss) for r in range(4)], axis=1).astype(BF)

    swp = np.zeros((128, 128), BF)
    swp[np.arange(128), (np.arange(128) + 64) % 128] = 1.0

    in_maps = []
    for c in range(8):
        b, p = divmod(c, 4)
        # per-core weight slices in on-chip tile layout
        wq_c = wq_p[:, 6 * p:6 * p + 6, :]          # [DIM, 6, 128]
        wk_c = wk_p[:, 2 * p:2 * p + 2, :]          # [DIM, 2, 128]
        wv_c = wv_r[:, 2 * p:2 * p + 2, :]          # [DIM, 2, 128]
        # w10[dt] = [128p, 24k, 128d] with DIM rows split as (k, p)
        wcat = np.concatenate([wq_c, wk_c, wv_c], axis=1)   # [DIM, 10, 128]
        w10 = np.ascontiguousarray(
            wcat.reshape(NKT, 128, NDT, HD).transpose(2, 1, 0, 3))
        # wo4[d] = [128p(dv), 6u, 128dd]; wo rows are (u, p)
        wo_c = wo_r[6 * p:6 * p + 6]                 # [6, 128, DIM]
        wo4 = np.ascontiguousarray(
            wo_c.reshape(QT_PER_CORE, HD, NKT, 128).transpose(2, 1, 0, 3)).astype(BF)
        # x4[j] = [128p, 24k, 512s]
        xT_b = x[b].T                                 # [DIM, S]
        x4 = np.ascontiguousarray(
            xT_b.reshape(NKT, 128, NJ, SW).transpose(2, 1, 0, 3)).astype(BF)
        in_maps.append({
            "x4": x4,
            "w10": w10,
            "wo4": wo4,
            "ropeC": ropeC,
            "ropeS": ropeS,
            "masks": masks,
            "swp": swp,
        })
    return in_maps


def gather(results):
    y = np.empty((B, S, DIM), np.float32)
    for b in range(B):
        acc = results[4 * b]["yT"].astype(np.float32)
        for p in range(1, 4):
            acc = acc + results[4 * b + p]["yT"].astype(np.float32)
        y[b] = acc.T
    return y


def kernel(x, wq, wk, wv, wo, freqs_cos, freqs_sin, **run_kwargs):
    nc = _get_nc()
    in_maps = make_in_maps(x, wq, wk, wv, wo, freqs_cos, freqs_sin)
    res = run_bass_kernel_spmd(nc, in_maps, core_ids=list(range(8)), **run_kwargs)
    out = gather(res.results)
    if run_kwargs:
        return out, res
    return out
